# revision 1
# baseline (speedup 1.0000x reference)
"""Trainium2 Bass kernel for CifNet conv-QKV self-attention.

Sharding: 8 cores = 4 (batch) x 2 (head-groups of 4 heads).
Each core computes, for its batch sample b and head-group g:
  - q/k/v = conv3x3(x, w{q,k,v}[g*256:(g+1)*256])   (256 out-channels = 4 heads)
  - per-head attention over hw=2304 positions (softmax without max-subtraction,
    denominator fused into the AV matmul via an appended ones-column on V^T)
  - partial o-conv: conv3x3(attn_out, wo[:, g*256:(g+1)*256])  -> [256, 2304] fp32
Host sums the two head-group partials per batch sample.

Convs are expressed as 9 shifted matmuls (one per tap) accumulating in PSUM,
with the input pre-padded to [C, 50, 50] on the host. All matmuls run in bf16
with fp32 PSUM accumulation (measured end-to-end rel-l2 error ~5.5e-3).
"""

from contextlib import ExitStack

import numpy as np
import ml_dtypes

# problem shape (hardcoded per contract)
B, C, H, W = 4, 256, 48, 48
HW = H * W              # 2304
NCORES = 8
RT = 8                  # output rows per spatial tile
NT = RT * W             # 384 columns per matmul
NROW = H // RT          # 6 spatial tiles
NKJ = HW // 128         # 18 key tiles
KJG = 3                 # kj tiles per exp group
NGRP = NKJ // KJG       # 6 groups

_cached = None


def _build():
    """Build and compile the per-core SPMD Bass program (cached)."""
    global _cached
    if _cached is not None:
        return _cached

    import concourse.bass as bass  # noqa: F401
    import concourse.tile as tile
    from concourse import bacc, mybir
    from concourse.masks import make_identity

    BF = mybir.dt.bfloat16
    F32 = mybir.dt.float32
    EXP = mybir.ActivationFunctionType.Exp

    nc = bacc.Bacc("TRN2", target_bir_lowering=False, debug=False)
    x_d = nc.dram_tensor("xpad", [2, 128, 50, 50], BF, kind="ExternalInput").ap()
    wqkv_d = nc.dram_tensor("wqkv", [3, 9, 2, 128, 256], BF, kind="ExternalInput").ap()
    wo_d = nc.dram_tensor("wo", [9, 2, 128, 256], BF, kind="ExternalInput").ap()
    out_d = nc.dram_tensor("out", [2, 128, HW], F32, kind="ExternalOutput").ap()

    with tile.TileContext(nc) as tc, ExitStack() as ctx:
        konst = ctx.enter_context(tc.tile_pool(name="konst", bufs=1))
        # identity for PE transpose; duplicated at base partitions 0 and 64 so
        # the transpose input/identity share a base partition
        ident = konst.tile([128, 64], BF, name="ident")
        make_identity(nc, ident[0:64, :])
        nc.sync.dma_start(ident[64:128, :], ident[0:64, :])

        x_sb = konst.tile([128, 2, 50, 50], BF, name="x_sb")
        wq_sb = konst.tile([128, 9, 2, 256], BF, name="wq_sb")
        wk_sb = konst.tile([128, 9, 2, 256], BF, name="wk_sb")
        wv_sb = konst.tile([128, 9, 2, 256], BF, name="wv_sb")
        wo_sb = konst.tile([128, 9, 2, 256], BF, name="wo_sb")
        q_sb = [konst.tile([128, HW], BF, name=f"q_sb{m}") for m in range(2)]
        k_sb = [konst.tile([128, HW], BF, name=f"k_sb{m}") for m in range(2)]
        v_sb = [konst.tile([128, HW], BF, name=f"v_sb{m}") for m in range(2)]
        # V^T per head: [kj within tile, kj tile, 65]; col 64 holds ones so the
        # AV matmul also produces the softmax denominator in psum row 64.
        vt_sb = [konst.tile([128, NKJ, 65], BF, name=f"vt_sb{h}") for h in range(4)]
        opad = [konst.tile([128, 50, 50], BF, name=f"opad{g}") for g in range(2)]

        # input DMAs
        for kg in range(2):
            nc.sync.dma_start(x_sb[:, kg], x_d[kg])
        for a, w_sb in enumerate((wq_sb, wk_sb, wv_sb)):
            for t in range(9):
                nc.sync.dma_start(w_sb[:, t], wqkv_d[a, t].rearrange("g p o -> p g o"))
        for t in range(9):
            nc.sync.dma_start(wo_sb[:, t], wo_d[t].rearrange("g p o -> p g o"))

        for h in range(4):
            nc.gpsimd.memset(vt_sb[h][:], 1.0)
        for g in range(2):
            nc.gpsimd.memset(opad[g][:], 0.0)

        # warm the ACT exp table during the DMA phase (one-time ~2.7us load)
        wrm = konst.tile([1, 8], F32, name="wrm")
        nc.gpsimd.memset(wrm[:], 0.0)
        nc.scalar.activation(wrm[:], wrm[:], EXP, scale=0.125)

        # ---------------- phase A: m0 convs + v-m0 transposes ----------------
        def conv_block(m, w_sb, dst, cpool, x_src):
            """One full conv output tile-row group: 18 accumulating MMs x 6 rowtiles."""
            ps = [cpool.tile([128, NT], F32, tag="cps", name="cps") for _ in range(NROW)]
            first = True
            for kg in range(2):
                for t in range(9):
                    ky, kx = t // 3, t % 3
                    lhsT = w_sb[:, t, kg, m * 128:(m + 1) * 128]
                    last = (kg == 1 and t == 8)
                    for r in range(NROW):
                        rhs = x_src[:, kg, r * RT + ky: r * RT + ky + RT, kx: kx + W]
                        nc.tensor.matmul(ps[r][:], lhsT, rhs, start=first, stop=last)
                    first = False
            for r in range(NROW):
                nc.vector.tensor_copy(dst[:, r * NT:(r + 1) * NT], ps[r][:])

        def transpose_unit(m, hh, kt, tpool, ttag="tps"):
            h = 2 * m + hh
            pt = tpool.tile([128, 64], BF, tag=ttag, name="tps")
            nc.tensor.transpose(
                pt[:],
                v_sb[m][64 * hh:64 * hh + 64, kt * 128:(kt + 1) * 128],
                ident[64 * hh:64 * hh + 64, :],
            )
            nc.vector.tensor_copy(vt_sb[h][:, kt, 0:64], pt[:])

        with tc.tile_pool(name="cpsum", bufs=6, space="PSUM") as cpsum, \
             tc.tile_pool(name="tpsum", bufs=2, space="PSUM") as tpsum:
            conv_block(0, wv_sb, v_sb[0], cpsum, x_sb)
            for hh in range(2):
                for kt in range(NKJ):
                    transpose_unit(0, hh, kt, tpsum)
            conv_block(0, wq_sb, q_sb[0], cpsum, x_sb)
            conv_block(0, wk_sb, k_sb[0], cpsum, x_sb)

        # ---------------- phases B/C/D: attention interleaved with m1 convs
        # and the o-conv, so the PE always has independent work and never
        # blips waiting on the ACT exp (which would throttle its clock).
        osum = [konst.tile([128, HW], F32, name=f"osum{mo}") for mo in range(2)]

        with tc.tile_pool(name="spsum", bufs=2, space="PSUM") as spsum, \
             tc.tile_pool(name="apsum", bufs=2, space="PSUM") as apsum, \
             tc.tile_pool(name="fpsum", bufs=2, space="PSUM") as fpsum, \
             tc.tile_pool(name="esb", bufs=4) as esb, \
             tc.tile_pool(name="osb", bufs=3) as osb, \
             tc.tile_pool(name="nsb", bufs=2) as nsb:

            def conv_row_unit(m, w_sb, dst, r):
                """One rowtile of a conv: 18 accumulating MMs into 1 psum bank."""
                ps = fpsum.tile([128, NT], F32, tag="fps", name="fps")
                first = True
                for kg in range(2):
                    for t in range(9):
                        ky, kx = t // 3, t % 3
                        lhsT = w_sb[:, t, kg, m * 128:(m + 1) * 128]
                        rhs = x_sb[:, kg, r * RT + ky: r * RT + ky + RT, kx: kx + W]
                        nc.tensor.matmul(ps[:], lhsT, rhs, start=first,
                                         stop=(kg == 1 and t == 8))
                        first = False
                nc.vector.tensor_copy(dst[:, r * NT:(r + 1) * NT], ps[:])

            def oconv_row_unit(mo, r, kg):
                """One rowtile of the o-conv for one input kgroup (9 taps)."""
                ps = fpsum.tile([128, NT], F32, tag="fps", name="fps")
                for t in range(9):
                    ky, kx = t // 3, t % 3
                    lhsT = wo_sb[:, t, kg, mo * 128:(mo + 1) * 128]
                    rhs = opad[kg][:, r * RT + ky: r * RT + ky + RT, kx: kx + W]
                    nc.tensor.matmul(ps[:], lhsT, rhs, start=(t == 0), stop=(t == 8))
                if kg == 0:
                    nc.vector.tensor_copy(osum[mo][:, r * NT:(r + 1) * NT], ps[:])
                else:
                    ot = osb.tile([128, NT], F32, tag="osb", name="osb")
                    nc.vector.tensor_tensor(
                        ot[:], ps[:], osum[mo][:, r * NT:(r + 1) * NT],
                        mybir.AluOpType.add,
                    )
                    nc.sync.dma_start(out_d[mo, :, r * NT:(r + 1) * NT], ot[:])

            def att_unit(m, qi, grp2):
                """Both heads / 2 kj tiles: 4 row-packed score MMs (adjacent
                (0,0)/(64,0) pairs overlap in the array), 2 exps, 4 AV MMs."""
                qsl = slice(qi * NT, (qi + 1) * NT)
                sp = [spsum.tile([128, 2, 512], F32, tag="sps", name="sps")
                      for _ in range(2)]
                for j in range(2):
                    kjt = grp2 * 2 + j
                    for hh in range(2):
                        nc.tensor.matmul(
                            sp[hh][:, j, 0:NT],
                            k_sb[m][64 * hh:64 * hh + 64, kjt * 128:(kjt + 1) * 128],
                            q_sb[m][64 * hh:64 * hh + 64, qsl],
                            start=True, stop=True,
                            tile_position=(64 * hh, 0),
                        )
                ets = []
                for hh in range(2):
                    et = esb.tile([128, 2, NT], BF, tag="et", name="et")
                    nc.scalar.activation(et[:], sp[hh][:, :, 0:NT], EXP, scale=0.125)
                    ets.append(et)
                for hh in range(2):
                    h = 2 * m + hh
                    for j in range(2):
                        kjt = grp2 * 2 + j
                        nc.tensor.matmul(
                            av_cur[hh][0:65, :], vt_sb[h][:, kjt, 0:65],
                            ets[hh][:, j, :],
                            start=(kjt == 0), stop=(kjt == NKJ - 1),
                        )

            def normalize(m, qi, hh):
                avf = nsb.tile([128, NT], F32, tag="avf", name="avf")
                nc.vector.tensor_copy(avf[0:65, :], av_cur[hh][0:65, :])
                dn = nsb.tile([1, NT], F32, tag="dn", name="dn")
                nc.sync.dma_start(dn[:], avf[64:65, :])
                rc = nsb.tile([1, NT], F32, tag="rc", name="rc")
                nc.vector.reciprocal_approx_fast(rc[:], dn[:])
                rb = nsb.tile([64, NT], F32, tag="rb", name="rb")
                nc.gpsimd.partition_broadcast(rb[:], rc[:])
                tmp = nsb.tile([64, NT], BF, tag="tmp", name="tmp")
                nc.vector.tensor_mul(tmp[:], avf[0:64, :], rb[:])
                dst = opad[m][64 * hh:64 * hh + 64, qi * RT + 1: qi * RT + RT + 1, 1:49]
                nc.sync.dma_start(dst, tmp[:].rearrange("p (r c) -> p r c", c=W))

            # filler list: PE-only work dripped into the attention stream, in
            # dependency order (v conv first, then its transposes, then q/k)
            fillers_b = (
                [lambda r=r: conv_row_unit(1, wv_sb, v_sb[1], r) for r in range(NROW)]
                + [lambda hh=hh, kt=kt: transpose_unit(1, hh, kt, fpsum, ttag="fps")
                   for hh in range(2) for kt in range(NKJ)]
                + [lambda r=r: conv_row_unit(1, wq_sb, q_sb[1], r) for r in range(NROW)]
                + [lambda r=r: conv_row_unit(1, wk_sb, k_sb[1], r) for r in range(NROW)]
            )

            def run_attention(m, fillers):
                """Emit all attention units for head-pair m, interspersing fillers."""
                fi = 0
                n_units = NROW * 9
                ui = 0
                for qi in range(NROW):
                    av_cur[0] = apsum.tile([128, NT], F32, tag="avps", name="avps")
                    av_cur[1] = apsum.tile([128, NT], F32, tag="avps", name="avps")
                    for grp2 in range(9):
                        att_unit(m, qi, grp2)
                        ui += 1
                        # drip PE-only work at a steady rate
                        while fi < len(fillers) and ui * len(fillers) >= (fi + 1) * n_units:
                            fillers[fi]()
                            fi += 1
                    for hh in range(2):
                        normalize(m, qi, hh)
                while fi < len(fillers):
                    fillers[fi]()
                    fi += 1

            av_cur = [None, None]
            run_attention(0, fillers_b)

            # phase C: attention m1, interleaved with o-conv kg0 partial pass
            fillers_c = [lambda mo=mo, r=r: oconv_row_unit(mo, r, 0)
                         for mo in range(2) for r in range(NROW)]
            run_attention(1, fillers_c)

            # phase D: o-conv kg1 pass + combine + store
            for mo in range(2):
                for r in range(NROW):
                    oconv_row_unit(mo, r, 1)

    nc.compile()
    _cached = nc
    return nc


def make_in_maps(hidden_states, wq, wk, wv, wo):
    """Shard + pre-transform full inputs into 8 per-core input dicts."""
    bf = ml_dtypes.bfloat16
    hidden_states = np.asarray(hidden_states, np.float32)
    in_maps = []
    for core in range(NCORES):
        b, g = core // 2, core % 2
        xp = np.zeros((C, 50, 50), np.float32)
        xp[:, 1:49, 1:49] = hidden_states[b]
        xpad = np.ascontiguousarray(xp.reshape(2, 128, 50, 50)).astype(bf)
        wstk = np.stack(
            [
                np.asarray(w, np.float32)[g * 256:(g + 1) * 256]
                .transpose(2, 3, 1, 0)
                .reshape(9, 2, 128, 256)
                for w in (wq, wk, wv)
            ]
        ).astype(bf)
        wog = (
            np.asarray(wo, np.float32)[:, g * 256:(g + 1) * 256]
            .transpose(2, 3, 1, 0)
            .reshape(9, 2, 128, 256)
            .astype(bf)
        )
        in_maps.append({"xpad": xpad, "wqkv": wstk, "wo": wog})
    return in_maps


def combine_outputs(per_core_outs):
    """Sum the two head-group partials per batch sample."""
    out = np.empty((B, C, H, W), np.float32)
    for b in range(B):
        acc = per_core_outs[2 * b].reshape(C, HW).astype(np.float32) + \
              per_core_outs[2 * b + 1].reshape(C, HW).astype(np.float32)
        out[b] = acc.reshape(C, H, W)
    return out


def kernel(hidden_states, wq, wk, wv, wo):
    from concourse.bass_utils import run_bass_kernel_spmd

    nc = _build()
    in_maps = make_in_maps(hidden_states, wq, wk, wv, wo)
    res = run_bass_kernel_spmd(nc, in_maps, core_ids=list(range(NCORES)))
    return combine_outputs([r["out"] for r in res.results])



# revision 6
# speedup vs baseline: 1.0846x; 1.0846x over previous
"""Trainium2 Bass kernel for CifNet conv-QKV self-attention.

Sharding: 8 cores = 4 (batch) x 2 (head-groups of 4 heads).
Each core computes, for its batch sample b and head-group g:
  - q/k/v = conv3x3(x, w{q,k,v}[g*256:(g+1)*256])   (256 out-channels = 4 heads)
  - per-head attention over hw=2304 positions (softmax without max-subtraction,
    denominator fused into the AV matmul via an appended ones-column on V^T)
  - partial o-conv: conv3x3(attn_out, wo[:, g*256:(g+1)*256])  -> [256, 2304] fp32
Host sums the two head-group partials per batch sample.

Convs are expressed as 9 shifted matmuls (one per tap) accumulating in PSUM,
with the input pre-padded to [C, 50, 50] on the host. All matmuls run in bf16
with fp32 PSUM accumulation.

Perf structure (v2):
  - All host->device tensors are partition-major so each input lands in one
    large contiguous-per-partition DMA (descriptor count ~100x lower than v1).
  - K is stored zero-padded per head (k0p/k1p with the other head's 64
    partitions zeroed) so the score matmuls are full 128x128-mode matmuls:
    no PE tile-config switches anywhere in the steady state (the 64-row
    score mode forced a PE drain at every 64<->128 transition, ~95ns each).
  - PE warmup matmuls run during the input DMA window so the HAM clock gate
    reaches 2.4 GHz before the first conv.
  - The o-conv kg1 pass is staggered into the m1 attention stream as its
    opad rows become available; outputs stream out per row-tile.
  - normalize() writes opad directly from the vector engine (strided AP)
    instead of a small-line DMA.
"""

from contextlib import ExitStack

import numpy as np
import ml_dtypes

# problem shape (hardcoded per contract)
B, C, H, W = 4, 256, 48, 48
HW = H * W              # 2304
NCORES = 8
RT = 8                  # output rows per spatial tile
NT = RT * W             # 384 columns per matmul
NROW = H // RT          # 6 spatial tiles
NKJ = HW // 128         # 18 key tiles

_cached = None


def _build():
    """Build and compile the per-core SPMD Bass program (cached)."""
    global _cached
    if _cached is not None:
        return _cached

    import concourse.bass as bass  # noqa: F401
    import concourse.tile as tile
    from concourse import bacc, mybir
    from concourse.masks import make_identity

    BF = mybir.dt.bfloat16
    F32 = mybir.dt.float32
    EXP = mybir.ActivationFunctionType.Exp

    nc = bacc.Bacc("TRN2", target_bir_lowering=False, debug=False)
    x_d = nc.dram_tensor("xpad", [128, 2, 50, 50], BF, kind="ExternalInput").ap()
    wqkv_d = nc.dram_tensor("wqkv", [128, 3, 9, 2, 256], BF, kind="ExternalInput").ap()
    wo_d = nc.dram_tensor("wo", [128, 9, 2, 256], BF, kind="ExternalInput").ap()
    out_d = nc.dram_tensor("out", [2, 128, HW], F32, kind="ExternalOutput").ap()

    with tile.TileContext(nc) as tc, ExitStack() as ctx:
        konst = ctx.enter_context(tc.tile_pool(name="konst", bufs=1))
        # identity for PE transpose; duplicated at base partitions 0 and 64 so
        # the transpose input/identity share a base partition
        ident = konst.tile([128, 64], BF, name="ident")
        make_identity(nc, ident[0:64, :])
        nc.sync.dma_start(ident[64:128, :], ident[0:64, :])

        x_sb = konst.tile([128, 2, 50, 50], BF, name="x_sb")
        wqkv_sb = konst.tile([128, 3, 9, 2, 256], BF, name="wqkv_sb")
        wo_sb = konst.tile([128, 9, 2, 256], BF, name="wo_sb")
        q_sb = [konst.tile([128, HW], BF, name=f"q_sb{m}") for m in range(2)]
        v_sb = [konst.tile([128, HW], BF, name=f"v_sb{m}") for m in range(2)]
        # K zero-padded per head: k0p has head hh=0 data in partitions 0:64 and
        # zeros in 64:128; k1p the reverse.  Score matmuls then contract over
        # the full 128 partitions (128x128 PE mode, no tile-config switches).
        k0p = [konst.tile([128, HW], BF, name=f"k0p{m}") for m in range(2)]
        k1p = [konst.tile([128, HW], BF, name=f"k1p{m}") for m in range(2)]
        # V^T per head: [kj within tile, kj tile, 65]; col 64 holds ones so the
        # AV matmul also produces the softmax denominator in psum row 64.
        vt_sb = [konst.tile([128, NKJ, 65], BF, name=f"vt_sb{h}") for h in range(4)]
        opad = [konst.tile([128, 50, 50], BF, name=f"opad{g}") for g in range(2)]
        osum = [konst.tile([128, HW], F32, name=f"osum{mo}") for mo in range(2)]
        wrm_sb = konst.tile([128, 512], BF, name="wrm_sb")

        # ---- input DMAs: big contiguous-per-partition transfers, spread over
        # the three DGE queues, ordered by first use (x+wv, wk, wq, wo).
        nc.sync.dma_start(x_sb[:, 0], x_d[:, 0])
        nc.scalar.dma_start(x_sb[:, 1], x_d[:, 1])
        nc.sync.dma_start(wqkv_sb[:, 2], wqkv_d[:, 2])    # wv
        nc.scalar.dma_start(wqkv_sb[:, 1], wqkv_d[:, 1])  # wk
        nc.sync.dma_start(wqkv_sb[:, 0], wqkv_d[:, 0])    # wq
        nc.gpsimd.dma_start(wo_sb[:], wo_d[:])            # wo (needed last)

        for h in range(4):
            nc.gpsimd.memset(vt_sb[h][:], 1.0)
        for g in range(2):
            nc.gpsimd.memset(opad[g][:], 0.0)
        for m in range(2):
            nc.vector.memset(k0p[m][64:128, :], 0.0)
            nc.vector.memset(k1p[m][0:64, :], 0.0)

        # warm the ACT exp table during the DMA phase (one-time ~2.7us load)
        wrm = konst.tile([1, 8], F32, name="wrm")
        nc.gpsimd.memset(wrm[:], 0.0)
        nc.scalar.activation(wrm[:], wrm[:], EXP, scale=0.125)

        # PE warmup: ~4us of matmuls on scratch data during the DMA window so
        # the HAM clock gate is at 2.4 GHz when the first conv issues.
        nc.vector.memset(wrm_sb[:], 0.0)
        with tc.tile_pool(name="wpsum", bufs=1, space="PSUM") as wpsum:
            wt = wpsum.tile([128, 512], F32, name="wt")
            NWARM = 12
            for i in range(NWARM):
                nc.tensor.matmul(wt[:], wrm_sb[:, 0:128], wrm_sb[:],
                                 start=(i == 0), stop=(i == NWARM - 1))

        WQ, WK, WV = 0, 1, 2

        def conv_lhsT(a, t, kg, m):
            return wqkv_sb[:, a, t, kg, m * 128:(m + 1) * 128]

        def copy_plain(dst):
            def w(r, ps):
                nc.vector.tensor_copy(dst[:, r * NT:(r + 1) * NT], ps[:])
            return w

        def copy_ksplit(m):
            def w(r, ps):
                sl = slice(r * NT, (r + 1) * NT)
                nc.vector.tensor_copy(k0p[m][0:64, sl], ps[0:64, :])
                nc.vector.tensor_copy(k1p[m][64:128, sl], ps[64:128, :])
            return w

        def conv_row(m, a, writer, r, pool, tag):
            """One rowtile of a qkv conv: 18 accumulating MMs into 1 psum bank."""
            ps = pool.tile([128, NT], F32, tag=tag, name=tag)
            first = True
            for kg in range(2):
                for t in range(9):
                    ky, kx = t // 3, t % 3
                    rhs = x_sb[:, kg, r * RT + ky: r * RT + ky + RT, kx: kx + W]
                    nc.tensor.matmul(ps[:], conv_lhsT(a, t, kg, m), rhs,
                                     start=first, stop=(kg == 1 and t == 8))
                    first = False
            writer(r, ps)

        def transpose_unit(m, hh, kt, tpool, ttag):
            h = 2 * m + hh
            pt = tpool.tile([128, 64], BF, tag=ttag, name=ttag)
            nc.tensor.transpose(
                pt[:],
                v_sb[m][64 * hh:64 * hh + 64, kt * 128:(kt + 1) * 128],
                ident[64 * hh:64 * hh + 64, :],
            )
            nc.vector.tensor_copy(vt_sb[h][:, kt, 0:64], pt[:])

        # ---------------- phase A: m0 convs + v-m0 transposes ----------------
        with tc.tile_pool(name="cpsum", bufs=6, space="PSUM") as cpsum, \
             tc.tile_pool(name="tpsum", bufs=2, space="PSUM") as tpsum:
            for r in range(NROW):
                conv_row(0, WV, copy_plain(v_sb[0]), r, cpsum, "cps")
            for hh in range(2):
                for kt in range(NKJ):
                    transpose_unit(0, hh, kt, tpsum, "tps")
            for r in range(NROW):
                conv_row(0, WK, copy_ksplit(0), r, cpsum, "cps")
            conv_row(0, WQ, copy_plain(q_sb[0]), 0, cpsum, "cps")

        # ---------------- phases B/C: attention interleaved with the rest ----
        with tc.tile_pool(name="spsum", bufs=2, space="PSUM") as spsum, \
             tc.tile_pool(name="apsum", bufs=2, space="PSUM") as apsum, \
             tc.tile_pool(name="fpsum", bufs=2, space="PSUM") as fpsum, \
             tc.tile_pool(name="esb", bufs=4) as esb, \
             tc.tile_pool(name="osb", bufs=3) as osb, \
             tc.tile_pool(name="nsb", bufs=2) as nsb:

            def oconv_row_unit(mo, r, kg):
                """One rowtile of the o-conv for one input kgroup (9 taps)."""
                ps = fpsum.tile([128, NT], F32, tag="fps", name="fps")
                for t in range(9):
                    ky, kx = t // 3, t % 3
                    lhsT = wo_sb[:, t, kg, mo * 128:(mo + 1) * 128]
                    rhs = opad[kg][:, r * RT + ky: r * RT + ky + RT, kx: kx + W]
                    nc.tensor.matmul(ps[:], lhsT, rhs, start=(t == 0), stop=(t == 8))
                if kg == 0:
                    nc.vector.tensor_copy(osum[mo][:, r * NT:(r + 1) * NT], ps[:])
                else:
                    ot = osb.tile([128, NT], F32, tag="osb", name="osb")
                    nc.vector.tensor_tensor(
                        ot[:], ps[:], osum[mo][:, r * NT:(r + 1) * NT],
                        mybir.AluOpType.add,
                    )
                    nc.sync.dma_start(out_d[mo, :, r * NT:(r + 1) * NT], ot[:])

            def att_unit(m, qi, grp2):
                """Both heads / 2 kj tiles: 4 score MMs (full 128-contraction
                against zero-padded K), 2 exps, 4 AV MMs."""
                qsl = slice(qi * NT, (qi + 1) * NT)
                sp = [spsum.tile([128, 2, 512], F32, tag="sps", name="sps")
                      for _ in range(2)]
                for j in range(2):
                    kjt = grp2 * 2 + j
                    ksl = slice(kjt * 128, (kjt + 1) * 128)
                    nc.tensor.matmul(sp[0][:, j, 0:NT], k0p[m][:, ksl],
                                     q_sb[m][:, qsl], start=True, stop=True)
                    nc.tensor.matmul(sp[1][:, j, 0:NT], k1p[m][:, ksl],
                                     q_sb[m][:, qsl], start=True, stop=True)
                ets = []
                for hh in range(2):
                    et = esb.tile([128, 2, NT], BF, tag="et", name="et")
                    nc.scalar.activation(et[:], sp[hh][:, :, 0:NT], EXP, scale=0.125)
                    ets.append(et)
                for hh in range(2):
                    h = 2 * m + hh
                    for j in range(2):
                        kjt = grp2 * 2 + j
                        nc.tensor.matmul(
                            av_cur[hh][0:65, :], vt_sb[h][:, kjt, 0:65],
                            ets[hh][:, j, :],
                            start=(kjt == 0), stop=(kjt == NKJ - 1),
                        )

            def normalize(m, qi, hh):
                avf = nsb.tile([128, NT], F32, tag="avf", name="avf")
                nc.vector.tensor_copy(avf[0:65, :], av_cur[hh][0:65, :])
                dn = nsb.tile([1, NT], F32, tag="dn", name="dn")
                nc.sync.dma_start(dn[:], avf[64:65, :])
                rc = nsb.tile([1, NT], F32, tag="rc", name="rc")
                nc.vector.reciprocal_approx_fast(rc[:], dn[:])
                rb = nsb.tile([64, NT], F32, tag="rb", name="rb")
                nc.gpsimd.partition_broadcast(rb[:], rc[:])
                dst = opad[m][64 * hh:64 * hh + 64,
                              qi * RT + 1: qi * RT + RT + 1, 1:49]
                nc.vector.tensor_tensor(
                    dst,
                    avf[0:64, :].rearrange("p (r c) -> p r c", c=W),
                    rb[:].rearrange("p (r c) -> p r c", c=W),
                    mybir.AluOpType.mult,
                )

            def run_attention(m, row_fillers, paced):
                """Emit all attention units for head-pair m.

                row_fillers: dict qi -> list of callables emitted at row start.
                paced: list of (mm_weight, callable) dripped across all units
                at a rate proportional to matmul count.
                """
                fi = 0
                n_units = NROW * 9
                total_w = sum(w for w, _ in paced) or 1
                done_w = 0
                ui = 0
                for qi in range(NROW):
                    for f in row_fillers.get(qi, ()):
                        f()
                    av_cur[0] = apsum.tile([128, NT], F32, tag="avps", name="avps")
                    av_cur[1] = apsum.tile([128, NT], F32, tag="avps", name="avps")
                    for grp2 in range(9):
                        att_unit(m, qi, grp2)
                        ui += 1
                        while fi < len(paced) and done_w * n_units < ui * total_w:
                            w, f = paced[fi]
                            f()
                            done_w += w
                            fi += 1
                    for hh in range(2):
                        normalize(m, qi, hh)
                while fi < len(paced):
                    paced[fi][1]()
                    fi += 1

            av_cur = [None, None]

            # --- m0 attention.  Row fillers: next q-m0 rowtile.  Paced: the
            # full m1 conv pipeline (v, transposes, k, q-r0).
            rowf_b = {qi: [lambda r=qi + 1: conv_row(0, WQ, copy_plain(q_sb[0]),
                                                     r, fpsum, "fps")]
                      for qi in range(NROW - 1)}
            paced_b = (
                [(18, lambda r=r: conv_row(1, WV, copy_plain(v_sb[1]), r,
                                           fpsum, "fps"))
                 for r in range(NROW)]
                + [(1, lambda hh=hh, kt=kt: transpose_unit(1, hh, kt,
                                                           fpsum, "fps"))
                   for hh in range(2) for kt in range(NKJ)]
                + [(18, lambda r=r: conv_row(1, WK, copy_ksplit(1), r,
                                             fpsum, "fps"))
                   for r in range(NROW)]
                + [(18, lambda: conv_row(1, WQ, copy_plain(q_sb[1]), 0,
                                         fpsum, "fps"))]
            )
            run_attention(0, rowf_b, paced_b)

            # --- m1 attention.  Row fillers: next q-m1 rowtile, plus the
            # o-conv kg1 rows whose opad[1] inputs completed two rows ago.
            # Paced: the o-conv kg0 pass (opad[0] is fully available).
            rowf_c = {}
            for qi in range(NROW - 1):
                rowf_c[qi] = [lambda r=qi + 1: conv_row(1, WQ, copy_plain(q_sb[1]),
                                                        r, fpsum, "fps")]
            for qi in range(2, NROW):
                rowf_c.setdefault(qi, []).extend(
                    [lambda mo=mo, r=qi - 2: oconv_row_unit(mo, r, 1)
                     for mo in range(2)]
                )
            # r-outer so both kg0 partials for row r are emitted well before
            # the kg1 pass for row r (rowf_c at qi=r+2) reads osum.
            paced_c = [(9, lambda mo=mo, r=r: oconv_row_unit(mo, r, 0))
                       for r in range(NROW) for mo in range(2)]
            run_attention(1, rowf_c, paced_c)

            # tail: the last two kg1 o-conv rows (need the final normalizes)
            for r in (NROW - 2, NROW - 1):
                for mo in range(2):
                    oconv_row_unit(mo, r, 1)

    nc.compile()
    _cached = nc
    return nc


def make_in_maps(hidden_states, wq, wk, wv, wo):
    """Shard + pre-transform full inputs into 8 per-core input dicts.

    All device tensors are partition-major so each DMA is one descriptor
    per partition with a large contiguous line.
    """
    bf = ml_dtypes.bfloat16
    hidden_states = np.asarray(hidden_states, np.float32)
    in_maps = []
    for core in range(NCORES):
        b, g = core // 2, core % 2
        xp = np.zeros((C, 50, 50), np.float32)
        xp[:, 1:49, 1:49] = hidden_states[b]
        # [2, 128, 50, 50] -> [128, 2, 50, 50]
        xpad = np.ascontiguousarray(
            xp.reshape(2, 128, 50, 50).transpose(1, 0, 2, 3)
        ).astype(bf)
        # [3, 9, 2, 128, 256] -> [128, 3, 9, 2, 256]
        wstk = np.stack(
            [
                np.asarray(w, np.float32)[g * 256:(g + 1) * 256]
                .transpose(2, 3, 1, 0)
                .reshape(9, 2, 128, 256)
                for w in (wq, wk, wv)
            ]
        )
        wstk = np.ascontiguousarray(wstk.transpose(3, 0, 1, 2, 4)).astype(bf)
        # [9, 2, 128, 256] -> [128, 9, 2, 256]
        wog = (
            np.asarray(wo, np.float32)[:, g * 256:(g + 1) * 256]
            .transpose(2, 3, 1, 0)
            .reshape(9, 2, 128, 256)
        )
        wog = np.ascontiguousarray(wog.transpose(2, 0, 1, 3)).astype(bf)
        in_maps.append({"xpad": xpad, "wqkv": wstk, "wo": wog})
    return in_maps


def combine_outputs(per_core_outs):
    """Sum the two head-group partials per batch sample."""
    out = np.empty((B, C, H, W), np.float32)
    for b in range(B):
        acc = per_core_outs[2 * b].reshape(C, HW).astype(np.float32) + \
              per_core_outs[2 * b + 1].reshape(C, HW).astype(np.float32)
        out[b] = acc.reshape(C, H, W)
    return out


def kernel(hidden_states, wq, wk, wv, wo):
    from concourse.bass_utils import run_bass_kernel_spmd

    nc = _build()
    in_maps = make_in_maps(hidden_states, wq, wk, wv, wo)
    res = run_bass_kernel_spmd(nc, in_maps, core_ids=list(range(NCORES)))
    return combine_outputs([r["out"] for r in res.results])


# revision 13
# speedup vs baseline: 1.1022x; 1.0162x over previous
"""Trainium2 Bass kernel for CifNet conv-QKV self-attention.

Sharding: 8 cores = 4 (batch) x 2 (head-groups of 4 heads).
Each core computes, for its batch sample b and head-group g:
  - q/k/v = conv3x3(x, w{q,k,v}[g*256:(g+1)*256])   (256 out-channels = 4 heads)
  - per-head attention over hw=2304 positions (softmax without max-subtraction,
    denominator fused into the AV matmul via an appended ones-column on V^T)
  - partial o-conv: conv3x3(attn_out, wo[:, g*256:(g+1)*256])  -> [256, 2304] fp32
Host sums the two head-group partials per batch sample.

Convs are expressed as 9 shifted matmuls (one per tap) accumulating in PSUM,
with the input pre-padded to [C, 50, 50] on the host. All matmuls run in bf16
with fp32 PSUM accumulation.

Perf structure (v2):
  - All host->device tensors are partition-major so each input lands in one
    large contiguous-per-partition DMA (descriptor count ~100x lower than v1).
  - K is stored zero-padded per head (k0p/k1p with the other head's 64
    partitions zeroed) so the score matmuls are full 128x128-mode matmuls:
    no PE tile-config switches anywhere in the steady state (the 64-row
    score mode forced a PE drain at every 64<->128 transition, ~95ns each).
  - PE warmup matmuls run during the input DMA window so the HAM clock gate
    reaches 2.4 GHz before the first conv.
  - The o-conv kg1 pass is staggered into the m1 attention stream as its
    opad rows become available; outputs stream out per row-tile.
  - normalize() writes opad directly from the vector engine (strided AP)
    instead of a small-line DMA.
"""

from contextlib import ExitStack

import numpy as np
import ml_dtypes

# problem shape (hardcoded per contract)
B, C, H, W = 4, 256, 48, 48
HW = H * W              # 2304
NCORES = 8
RT = 8                  # output rows per spatial tile
NT = RT * W             # 384 columns per matmul
NROW = H // RT          # 6 spatial tiles
NKJ = HW // 128         # 18 key tiles

_cached = None


def _build():
    """Build and compile the per-core SPMD Bass program (cached)."""
    global _cached
    if _cached is not None:
        return _cached

    import concourse.bass as bass  # noqa: F401
    import concourse.tile as tile
    from concourse import bacc, mybir
    from concourse.masks import make_identity

    BF = mybir.dt.bfloat16
    F32 = mybir.dt.float32
    EXP = mybir.ActivationFunctionType.Exp

    nc = bacc.Bacc("TRN2", target_bir_lowering=False, debug=False)
    x_d = nc.dram_tensor("xpad", [128, 2, 50, 50], BF, kind="ExternalInput").ap()
    # m-major weight layout so each (matrix, m-half) is one contiguous DMA
    wqkv_d = nc.dram_tensor(
        "wqkv", [128, 3, 2, 9, 2, 128], BF, kind="ExternalInput").ap()
    wo_d = nc.dram_tensor("wo", [128, 9, 2, 256], BF, kind="ExternalInput").ap()
    out_d = nc.dram_tensor("out", [2, 128, HW], F32, kind="ExternalOutput").ap()

    with tile.TileContext(nc) as tc, ExitStack() as ctx:
        konst = ctx.enter_context(tc.tile_pool(name="konst", bufs=1))
        # identity for PE transpose; duplicated at base partitions 0 and 64 so
        # the transpose input/identity share a base partition
        ident = konst.tile([128, 64], BF, name="ident")

        x_sb = konst.tile([128, 2, 50, 50], BF, name="x_sb")
        wqkv_sb = konst.tile([128, 3, 2, 9, 2, 128], BF, name="wqkv_sb")
        wo_sb = konst.tile([128, 9, 2, 256], BF, name="wo_sb")
        q_sb = [konst.tile([128, HW], BF, name=f"q_sb{m}") for m in range(2)]
        v_sb = [konst.tile([128, HW], BF, name=f"v_sb{m}") for m in range(2)]
        # K zero-padded per head: k0p has head hh=0 data in partitions 0:64 and
        # zeros in 64:128; k1p the reverse.  Score matmuls then contract over
        # the full 128 partitions (128x128 PE mode, no tile-config switches).
        k0p = [konst.tile([128, HW], BF, name=f"k0p{m}") for m in range(2)]
        k1p = [konst.tile([128, HW], BF, name=f"k1p{m}") for m in range(2)]
        # V^T per head: [kj within tile, kj tile, 65]; col 64 holds ones so the
        # AV matmul also produces the softmax denominator in psum row 64.
        vt_sb = [konst.tile([128, NKJ, 65], BF, name=f"vt_sb{h}") for h in range(4)]
        opad = [konst.tile([128, 50, 50], BF, name=f"opad{g}") for g in range(2)]
        osum = [konst.tile([128, HW], F32, name=f"osum{mo}") for mo in range(2)]
        wrm_sb = konst.tile([128, 512], BF, name="wrm_sb")

        # ---- input DMAs.  Per-queue bandwidth is ~110-130 GB/s, so the bytes
        # the first conv waits on (x + wv-m0) are spread across all three DGE
        # queues; everything else follows in order of first use.
        WQ, WK, WV = 0, 1, 2
        nc.gpsimd.dma_start(wqkv_sb[:, WV, 0], wqkv_d[:, WV, 0])  # wv-m0
        nc.sync.dma_start(x_sb[:, 0], x_d[:, 0])
        nc.scalar.dma_start(x_sb[:, 1], x_d[:, 1])
        nc.sync.dma_start(wqkv_sb[:, WK, 0], wqkv_d[:, WK, 0])    # wk-m0
        nc.sync.dma_start(wqkv_sb[:, WQ, 0], wqkv_d[:, WQ, 0])    # wq-m0
        nc.sync.dma_start(wqkv_sb[:, WV, 1], wqkv_d[:, WV, 1])    # wv-m1
        nc.sync.dma_start(wqkv_sb[:, WK, 1], wqkv_d[:, WK, 1])    # wk-m1
        nc.sync.dma_start(wqkv_sb[:, WQ, 1], wqkv_d[:, WQ, 1])    # wq-m1
        nc.gpsimd.dma_start(wo_sb[:], wo_d[:])                    # wo (last use)

        # PE warmup scratch first on the vector queue so warmup matmuls can
        # start as soon as the engines come up.
        nc.vector.memset(wrm_sb[:], 0.0)
        for m in range(2):
            nc.vector.memset(k0p[m][64:128, :], 0.0)
            nc.vector.memset(k1p[m][0:64, :], 0.0)

        make_identity(nc, ident[0:64, :])
        # dup at base partition 64 (transpose input/identity share a base
        # partition); scalar queue is free after x-kg1 so this lands early
        nc.scalar.dma_start(ident[64:128, :], ident[0:64, :])
        for h in range(4):
            nc.gpsimd.memset(vt_sb[h][:], 1.0)
        for g in range(2):
            nc.gpsimd.memset(opad[g][:], 0.0)

        # warm the ACT exp table during the DMA phase (one-time ~2.7us load)
        wrm = konst.tile([1, 8], F32, name="wrm")
        nc.gpsimd.memset(wrm[:], 0.0)
        nc.scalar.activation(wrm[:], wrm[:], EXP, scale=0.125)

        # PE warmup: ~4us of matmuls on scratch data during the DMA window so
        # the HAM clock gate is at 2.4 GHz when the first conv issues.
        with tc.tile_pool(name="wpsum", bufs=1, space="PSUM") as wpsum:
            wt = wpsum.tile([128, 512], F32, name="wt")
            NWARM = 12
            for i in range(NWARM):
                nc.tensor.matmul(wt[:], wrm_sb[:, 0:128], wrm_sb[:],
                                 start=(i == 0), stop=(i == NWARM - 1))

        def conv_lhsT(a, t, kg, m):
            return wqkv_sb[:, a, m, t, kg, :]

        def copy_plain(dst):
            def w(r, ps):
                nc.vector.tensor_copy(dst[:, r * NT:(r + 1) * NT], ps[:])
            return w

        def copy_ksplit(m):
            def w(r, ps):
                sl = slice(r * NT, (r + 1) * NT)
                nc.vector.tensor_copy(k0p[m][0:64, sl], ps[0:64, :])
                nc.vector.tensor_copy(k1p[m][64:128, sl], ps[64:128, :])
            return w

        def conv_row(m, a, writer, r, pool, tag):
            """One rowtile of a qkv conv: 18 accumulating MMs into 1 psum bank."""
            ps = pool.tile([128, NT], F32, tag=tag, name=tag)
            first = True
            for kg in range(2):
                for t in range(9):
                    ky, kx = t // 3, t % 3
                    rhs = x_sb[:, kg, r * RT + ky: r * RT + ky + RT, kx: kx + W]
                    nc.tensor.matmul(ps[:], conv_lhsT(a, t, kg, m), rhs,
                                     start=first, stop=(kg == 1 and t == 8))
                    first = False
            writer(r, ps)

        def transpose_unit(m, hh, kt, tpool, ttag):
            h = 2 * m + hh
            pt = tpool.tile([128, 64], BF, tag=ttag, name=ttag)
            nc.tensor.transpose(
                pt[:],
                v_sb[m][64 * hh:64 * hh + 64, kt * 128:(kt + 1) * 128],
                ident[64 * hh:64 * hh + 64, :],
            )
            nc.vector.tensor_copy(vt_sb[h][:, kt, 0:64], pt[:])

        # ---------------- phase A: m0 convs + v-m0 transposes ----------------
        with tc.tile_pool(name="cpsum", bufs=6, space="PSUM") as cpsum, \
             tc.tile_pool(name="tpsum", bufs=2, space="PSUM") as tpsum:
            for r in range(NROW):
                conv_row(0, WV, copy_plain(v_sb[0]), r, cpsum, "cps")
            for hh in range(2):
                for kt in range(NKJ):
                    transpose_unit(0, hh, kt, tpsum, "tps")
            for r in range(NROW):
                conv_row(0, WK, copy_ksplit(0), r, cpsum, "cps")
            conv_row(0, WQ, copy_plain(q_sb[0]), 0, cpsum, "cps")

        # ---------------- phases B/C: attention interleaved with the rest ----
        with tc.tile_pool(name="spsum", bufs=2, space="PSUM") as spsum, \
             tc.tile_pool(name="apsum", bufs=2, space="PSUM") as apsum, \
             tc.tile_pool(name="fpsum", bufs=2, space="PSUM") as fpsum, \
             tc.tile_pool(name="esb", bufs=4) as esb, \
             tc.tile_pool(name="osb", bufs=3) as osb, \
             tc.tile_pool(name="nsb", bufs=2) as nsb:

            def oconv_row_unit(mo, r, kg):
                """One rowtile of the o-conv for one input kgroup (9 taps)."""
                ps = fpsum.tile([128, NT], F32, tag="fps", name="fps")
                for t in range(9):
                    ky, kx = t // 3, t % 3
                    lhsT = wo_sb[:, t, kg, mo * 128:(mo + 1) * 128]
                    rhs = opad[kg][:, r * RT + ky: r * RT + ky + RT, kx: kx + W]
                    nc.tensor.matmul(ps[:], lhsT, rhs, start=(t == 0), stop=(t == 8))
                if kg == 0:
                    nc.vector.tensor_copy(osum[mo][:, r * NT:(r + 1) * NT], ps[:])
                else:
                    ot = osb.tile([128, NT], F32, tag="osb", name="osb")
                    nc.vector.tensor_tensor(
                        ot[:], ps[:], osum[mo][:, r * NT:(r + 1) * NT],
                        mybir.AluOpType.add,
                    )
                    nc.sync.dma_start(out_d[mo, :, r * NT:(r + 1) * NT], ot[:])

            def att_unit(m, qi, grp2):
                """Both heads / 2 kj tiles: 4 score MMs (full 128-contraction
                against zero-padded K), 2 exps, 4 AV MMs."""
                qsl = slice(qi * NT, (qi + 1) * NT)
                sp = [spsum.tile([128, 2, 512], F32, tag="sps", name="sps")
                      for _ in range(2)]
                for j in range(2):
                    kjt = grp2 * 2 + j
                    ksl = slice(kjt * 128, (kjt + 1) * 128)
                    nc.tensor.matmul(sp[0][:, j, 0:NT], k0p[m][:, ksl],
                                     q_sb[m][:, qsl], start=True, stop=True)
                    nc.tensor.matmul(sp[1][:, j, 0:NT], k1p[m][:, ksl],
                                     q_sb[m][:, qsl], start=True, stop=True)
                ets = []
                for hh in range(2):
                    et = esb.tile([128, 2, NT], BF, tag="et", name="et")
                    nc.scalar.activation(et[:], sp[hh][:, :, 0:NT], EXP, scale=0.125)
                    ets.append(et)
                for hh in range(2):
                    h = 2 * m + hh
                    for j in range(2):
                        kjt = grp2 * 2 + j
                        nc.tensor.matmul(
                            av_cur[hh][0:65, :], vt_sb[h][:, kjt, 0:65],
                            ets[hh][:, j, :],
                            start=(kjt == 0), stop=(kjt == NKJ - 1),
                        )

            def normalize(m, qi, hh):
                avf = nsb.tile([128, NT], F32, tag="avf", name="avf")
                nc.vector.tensor_copy(avf[0:65, :], av_cur[hh][0:65, :])
                dn = nsb.tile([1, NT], F32, tag="dn", name="dn")
                nc.sync.dma_start(dn[:], avf[64:65, :])
                rc = nsb.tile([1, NT], F32, tag="rc", name="rc")
                nc.vector.reciprocal_approx_fast(rc[:], dn[:])
                rb = nsb.tile([64, NT], F32, tag="rb", name="rb")
                nc.gpsimd.partition_broadcast(rb[:], rc[:])
                dst = opad[m][64 * hh:64 * hh + 64,
                              qi * RT + 1: qi * RT + RT + 1, 1:49]
                nc.vector.tensor_tensor(
                    dst,
                    avf[0:64, :].rearrange("p (r c) -> p r c", c=W),
                    rb[:].rearrange("p (r c) -> p r c", c=W),
                    mybir.AluOpType.mult,
                )

            def run_attention(m, row_fillers, paced):
                """Emit all attention units for head-pair m.

                row_fillers: dict qi -> list of callables emitted at row start.
                paced: list of (mm_weight, callable) dripped across all units
                at a rate proportional to matmul count.
                """
                fi = 0
                n_units = NROW * 9
                total_w = sum(w for w, _ in paced) or 1
                done_w = 0
                ui = 0
                for qi in range(NROW):
                    for f in row_fillers.get(qi, ()):
                        f()
                    av_cur[0] = apsum.tile([128, NT], F32, tag="avps", name="avps")
                    av_cur[1] = apsum.tile([128, NT], F32, tag="avps", name="avps")
                    for grp2 in range(9):
                        att_unit(m, qi, grp2)
                        ui += 1
                        while fi < len(paced) and done_w * n_units < ui * total_w:
                            w, f = paced[fi]
                            f()
                            done_w += w
                            fi += 1
                    for hh in range(2):
                        normalize(m, qi, hh)
                while fi < len(paced):
                    paced[fi][1]()
                    fi += 1

            av_cur = [None, None]

            # --- m0 attention.  Row fillers: next q-m0 rowtile.  Paced: the
            # full m1 conv pipeline (v, transposes, k, q-r0).
            rowf_b = {qi: [lambda r=qi + 1: conv_row(0, WQ, copy_plain(q_sb[0]),
                                                     r, fpsum, "fps")]
                      for qi in range(NROW - 1)}
            paced_b = (
                [(18, lambda r=r: conv_row(1, WV, copy_plain(v_sb[1]), r,
                                           fpsum, "fps"))
                 for r in range(NROW)]
                # transposes in contiguous batches of 9: the PE transpose mode
                # switch costs a drain, so don't drip them singly
                + [(9, lambda hh=hh, k0=k0: [
                        transpose_unit(1, hh, kt, fpsum, "fps")
                        for kt in range(k0, k0 + 9)])
                   for hh in range(2) for k0 in (0, 9)]
                + [(18, lambda r=r: conv_row(1, WK, copy_ksplit(1), r,
                                             fpsum, "fps"))
                   for r in range(NROW)]
                + [(18, lambda: conv_row(1, WQ, copy_plain(q_sb[1]), 0,
                                         fpsum, "fps"))]
            )
            run_attention(0, rowf_b, paced_b)

            # --- m1 attention.  Row fillers: next q-m1 rowtile, plus the
            # o-conv kg1 rows whose opad[1] inputs completed two rows ago.
            # Paced: the o-conv kg0 pass (opad[0] is fully available).
            rowf_c = {}
            for qi in range(NROW - 1):
                rowf_c[qi] = [lambda r=qi + 1: conv_row(1, WQ, copy_plain(q_sb[1]),
                                                        r, fpsum, "fps")]
            for qi in range(2, NROW):
                rowf_c.setdefault(qi, []).extend(
                    [lambda mo=mo, r=qi - 2: oconv_row_unit(mo, r, 1)
                     for mo in range(2)]
                )
            # r-outer so both kg0 partials for row r are emitted well before
            # the kg1 pass for row r (rowf_c at qi=r+2) reads osum.
            paced_c = [(9, lambda mo=mo, r=r: oconv_row_unit(mo, r, 0))
                       for r in range(NROW) for mo in range(2)]
            run_attention(1, rowf_c, paced_c)

            # tail: the last two kg1 o-conv rows (need the final normalizes)
            for r in (NROW - 2, NROW - 1):
                for mo in range(2):
                    oconv_row_unit(mo, r, 1)

    nc.compile()
    _cached = nc
    return nc


def make_in_maps(hidden_states, wq, wk, wv, wo):
    """Shard + pre-transform full inputs into 8 per-core input dicts.

    All device tensors are partition-major so each DMA is one descriptor
    per partition with a large contiguous line.
    """
    bf = ml_dtypes.bfloat16
    hidden_states = np.asarray(hidden_states, np.float32)
    in_maps = []
    for core in range(NCORES):
        b, g = core // 2, core % 2
        xp = np.zeros((C, 50, 50), np.float32)
        xp[:, 1:49, 1:49] = hidden_states[b]
        # [2, 128, 50, 50] -> [128, 2, 50, 50]
        xpad = np.ascontiguousarray(
            xp.reshape(2, 128, 50, 50).transpose(1, 0, 2, 3)
        ).astype(bf)
        # per matrix: [9, 2, 128p, 256o] -> [128p, 2m, 9, 2, 128o]; stack to
        # [128, 3, 2, 9, 2, 128]
        wstk = np.stack(
            [
                np.asarray(w, np.float32)[g * 256:(g + 1) * 256]
                .transpose(2, 3, 1, 0)
                .reshape(9, 2, 128, 2, 128)
                .transpose(2, 3, 0, 1, 4)
                for w in (wq, wk, wv)
            ],
            axis=1,
        )
        wstk = np.ascontiguousarray(wstk).astype(bf)
        # [9, 2, 128, 256] -> [128, 9, 2, 256]
        wog = (
            np.asarray(wo, np.float32)[:, g * 256:(g + 1) * 256]
            .transpose(2, 3, 1, 0)
            .reshape(9, 2, 128, 256)
        )
        wog = np.ascontiguousarray(wog.transpose(2, 0, 1, 3)).astype(bf)
        in_maps.append({"xpad": xpad, "wqkv": wstk, "wo": wog})
    return in_maps


def combine_outputs(per_core_outs):
    """Sum the two head-group partials per batch sample."""
    out = np.empty((B, C, H, W), np.float32)
    for b in range(B):
        acc = per_core_outs[2 * b].reshape(C, HW).astype(np.float32) + \
              per_core_outs[2 * b + 1].reshape(C, HW).astype(np.float32)
        out[b] = acc.reshape(C, H, W)
    return out


def kernel(hidden_states, wq, wk, wv, wo):
    from concourse.bass_utils import run_bass_kernel_spmd

    nc = _build()
    in_maps = make_in_maps(hidden_states, wq, wk, wv, wo)
    res = run_bass_kernel_spmd(nc, in_maps, core_ids=list(range(NCORES)))
    return combine_outputs([r["out"] for r in res.results])


# revision 18
# speedup vs baseline: 1.1286x; 1.0239x over previous
"""Trainium2 Bass kernel for CifNet conv-QKV self-attention.

Sharding: 8 cores = 4 (batch) x 2 (head-groups of 4 heads).
Each core computes, for its batch sample b and head-group g:
  - q/k/v = conv3x3(x, w{q,k,v}[g*256:(g+1)*256])   (256 out-channels = 4 heads)
  - per-head attention over hw=2304 positions (softmax without max-subtraction,
    denominator fused into the AV matmul via an appended ones-column on V^T)
  - partial o-conv: conv3x3(attn_out, wo[:, g*256:(g+1)*256])  -> [256, 2304] fp32
Host sums the two head-group partials per batch sample.

Convs are expressed as 9 shifted matmuls (one per tap) accumulating in PSUM,
with the input pre-padded to [C, 50, 50] on the host. All matmuls run in bf16
with fp32 PSUM accumulation.

Perf structure (v2):
  - All host->device tensors are partition-major so each input lands in one
    large contiguous-per-partition DMA (descriptor count ~100x lower than v1).
  - K is stored zero-padded per head (k0p/k1p with the other head's 64
    partitions zeroed) so the score matmuls are full 128x128-mode matmuls:
    no PE tile-config switches anywhere in the steady state (the 64-row
    score mode forced a PE drain at every 64<->128 transition, ~95ns each).
  - PE warmup matmuls run during the input DMA window so the HAM clock gate
    reaches 2.4 GHz before the first conv.
  - The o-conv kg1 pass is staggered into the m1 attention stream as its
    opad rows become available; outputs stream out per row-tile.
  - normalize() writes opad directly from the vector engine (strided AP)
    instead of a small-line DMA.
"""

from contextlib import ExitStack

import numpy as np
import ml_dtypes

# problem shape (hardcoded per contract)
B, C, H, W = 4, 256, 48, 48
HW = H * W              # 2304
NCORES = 8
RT = 8                  # output rows per spatial tile
NT = RT * W             # 384 columns per matmul
NROW = H // RT          # 6 spatial tiles
NKJ = HW // 128         # 18 key tiles

_cached = None


def _build():
    """Build and compile the per-core SPMD Bass program (cached)."""
    global _cached
    if _cached is not None:
        return _cached

    import concourse.bass as bass  # noqa: F401
    import concourse.tile as tile
    from concourse import bacc, mybir
    from concourse.masks import make_identity

    BF = mybir.dt.bfloat16
    F32 = mybir.dt.float32
    EXP = mybir.ActivationFunctionType.Exp

    nc = bacc.Bacc("TRN2", target_bir_lowering=False, debug=False)
    x_d = nc.dram_tensor("xpad", [128, 2, 50, 50], BF, kind="ExternalInput").ap()
    # m-major weight layout so each (matrix, m-half) is one contiguous DMA
    wqkv_d = nc.dram_tensor(
        "wqkv", [128, 3, 2, 9, 2, 128], BF, kind="ExternalInput").ap()
    wo_d = nc.dram_tensor("wo", [128, 9, 2, 256], BF, kind="ExternalInput").ap()
    out_d = nc.dram_tensor("out", [2, 128, HW], F32, kind="ExternalOutput").ap()

    with tile.TileContext(nc) as tc, ExitStack() as ctx:
        konst = ctx.enter_context(tc.tile_pool(name="konst", bufs=1))
        # identity for PE transpose; duplicated at base partitions 0 and 64 so
        # the transpose input/identity share a base partition
        ident = konst.tile([128, 64], BF, name="ident")

        x_sb = konst.tile([128, 2, 50, 50], BF, name="x_sb")
        wqkv_sb = konst.tile([128, 3, 2, 9, 2, 128], BF, name="wqkv_sb")
        wo_sb = konst.tile([128, 9, 2, 256], BF, name="wo_sb")
        q_sb = [konst.tile([128, HW], BF, name=f"q_sb{m}") for m in range(2)]
        v_sb = [konst.tile([128, HW], BF, name=f"v_sb{m}") for m in range(2)]
        # K zero-padded per head: k0p has head hh=0 data in partitions 0:64 and
        # zeros in 64:128; k1p the reverse.  Score matmuls then contract over
        # the full 128 partitions (128x128 PE mode, no tile-config switches).
        k0p = [konst.tile([128, HW], BF, name=f"k0p{m}") for m in range(2)]
        k1p = [konst.tile([128, HW], BF, name=f"k1p{m}") for m in range(2)]
        # V^T per head: [kj within tile, kj tile, 65]; col 64 holds ones so the
        # AV matmul also produces the softmax denominator in psum row 64.
        vt_sb = [konst.tile([128, NKJ, 65], BF, name=f"vt_sb{h}") for h in range(4)]
        opad = [konst.tile([128, 50, 50], BF, name=f"opad{g}") for g in range(2)]
        osum = [konst.tile([128, HW], F32, name=f"osum{mo}") for mo in range(2)]
        wrm_sb = konst.tile([128, 512], BF, name="wrm_sb")

        # ---- input DMAs.  HWDGE queues (sync/scalar) run ~130 GB/s each, the
        # gpsimd SWDGE queue ~63 GB/s.  The first conv row waits on x rows
        # 0-17 + wv-m0, so exactly those bytes are split across the two fast
        # queues first; m1 weights + wo (needed much later) go to gpsimd.
        WQ, WK, WV = 0, 1, 2
        nc.sync.dma_start(x_sb[:, 0, 0:18], x_d[:, 0, 0:18])
        nc.scalar.dma_start(x_sb[:, 1, 0:18], x_d[:, 1, 0:18])
        nc.sync.dma_start(wqkv_sb[:, WV, 0, 0:4], wqkv_d[:, WV, 0, 0:4])
        nc.scalar.dma_start(wqkv_sb[:, WV, 0, 4:9], wqkv_d[:, WV, 0, 4:9])
        nc.sync.dma_start(x_sb[:, 0, 18:50], x_d[:, 0, 18:50])
        nc.scalar.dma_start(x_sb[:, 1, 18:50], x_d[:, 1, 18:50])
        nc.sync.dma_start(wqkv_sb[:, WK, 0], wqkv_d[:, WK, 0])    # wk-m0
        nc.scalar.dma_start(wqkv_sb[:, WQ, 0], wqkv_d[:, WQ, 0])  # wq-m0
        nc.gpsimd.dma_start(wqkv_sb[:, WV, 1], wqkv_d[:, WV, 1])  # wv-m1
        nc.gpsimd.dma_start(wqkv_sb[:, WK, 1], wqkv_d[:, WK, 1])  # wk-m1
        nc.gpsimd.dma_start(wqkv_sb[:, WQ, 1], wqkv_d[:, WQ, 1])  # wq-m1
        nc.gpsimd.dma_start(wo_sb[:], wo_d[:])                    # wo (last use)

        # PE warmup scratch first on the vector queue so warmup matmuls can
        # start as soon as the engines come up.
        nc.vector.memset(wrm_sb[:], 0.0)
        for m in range(2):
            nc.vector.memset(k0p[m][64:128, :], 0.0)
            nc.vector.memset(k1p[m][0:64, :], 0.0)

        make_identity(nc, ident[0:64, :])
        # dup at base partition 64 (transpose input/identity share a base
        # partition); scalar queue drains early so this lands well before use
        nc.scalar.dma_start(ident[64:128, :], ident[0:64, :])
        for h in range(4):
            nc.gpsimd.memset(vt_sb[h][:], 1.0)
        for g in range(2):
            nc.gpsimd.memset(opad[g][:], 0.0)

        # warm the ACT exp table during the DMA phase (one-time ~2.7us load)
        wrm = konst.tile([1, 8], F32, name="wrm")
        nc.gpsimd.memset(wrm[:], 0.0)
        nc.scalar.activation(wrm[:], wrm[:], EXP, scale=0.125)

        # PE warmup: ~4us of matmuls on scratch data during the DMA window so
        # the HAM clock gate is at 2.4 GHz when the first conv issues.
        with tc.tile_pool(name="wpsum", bufs=1, space="PSUM") as wpsum:
            wt = wpsum.tile([128, 512], F32, name="wt")
            NWARM = 7
            for i in range(NWARM):
                nc.tensor.matmul(wt[:], wrm_sb[:, 0:128], wrm_sb[:],
                                 start=(i == 0), stop=(i == NWARM - 1))

        def conv_lhsT(a, t, kg, m):
            return wqkv_sb[:, a, m, t, kg, :]

        def copy_plain(dst):
            def w(r, ps):
                nc.vector.tensor_copy(dst[:, r * NT:(r + 1) * NT], ps[:])
            return w

        def copy_ksplit(m):
            def w(r, ps):
                sl = slice(r * NT, (r + 1) * NT)
                nc.vector.tensor_copy(k0p[m][0:64, sl], ps[0:64, :])
                nc.vector.tensor_copy(k1p[m][64:128, sl], ps[64:128, :])
            return w

        def conv_row(m, a, writer, r, pool, tag):
            """One rowtile of a qkv conv: 18 accumulating MMs into 1 psum bank."""
            ps = pool.tile([128, NT], F32, tag=tag, name=tag)
            first = True
            for kg in range(2):
                for t in range(9):
                    ky, kx = t // 3, t % 3
                    rhs = x_sb[:, kg, r * RT + ky: r * RT + ky + RT, kx: kx + W]
                    nc.tensor.matmul(ps[:], conv_lhsT(a, t, kg, m), rhs,
                                     start=first, stop=(kg == 1 and t == 8))
                    first = False
            writer(r, ps)

        def transpose_unit(m, hh, kt, tpool, ttag):
            h = 2 * m + hh
            pt = tpool.tile([128, 64], BF, tag=ttag, name=ttag)
            nc.tensor.transpose(
                pt[:],
                v_sb[m][64 * hh:64 * hh + 64, kt * 128:(kt + 1) * 128],
                ident[64 * hh:64 * hh + 64, :],
            )
            nc.vector.tensor_copy(vt_sb[h][:, kt, 0:64], pt[:])

        # ---------------- phase A: m0 convs + v-m0 transposes ----------------
        with tc.tile_pool(name="cpsum", bufs=6, space="PSUM") as cpsum, \
             tc.tile_pool(name="tpsum", bufs=2, space="PSUM") as tpsum:
            for r in range(NROW):
                conv_row(0, WV, copy_plain(v_sb[0]), r, cpsum, "cps")
            for hh in range(2):
                for kt in range(NKJ):
                    transpose_unit(0, hh, kt, tpsum, "tps")
            for r in range(NROW):
                conv_row(0, WK, copy_ksplit(0), r, cpsum, "cps")
            conv_row(0, WQ, copy_plain(q_sb[0]), 0, cpsum, "cps")

        # ---------------- phases B/C: attention interleaved with the rest ----
        with tc.tile_pool(name="spsum", bufs=2, space="PSUM") as spsum, \
             tc.tile_pool(name="apsum", bufs=2, space="PSUM") as apsum, \
             tc.tile_pool(name="fpsum", bufs=2, space="PSUM") as fpsum, \
             tc.tile_pool(name="esb", bufs=4) as esb, \
             tc.tile_pool(name="osb", bufs=3) as osb, \
             tc.tile_pool(name="nsb", bufs=2) as nsb:

            def oconv_row_unit(mo, r, kg):
                """One rowtile of the o-conv for one input kgroup (9 taps)."""
                ps = fpsum.tile([128, NT], F32, tag="fps", name="fps")
                for t in range(9):
                    ky, kx = t // 3, t % 3
                    lhsT = wo_sb[:, t, kg, mo * 128:(mo + 1) * 128]
                    rhs = opad[kg][:, r * RT + ky: r * RT + ky + RT, kx: kx + W]
                    nc.tensor.matmul(ps[:], lhsT, rhs, start=(t == 0), stop=(t == 8))
                if kg == 0:
                    nc.vector.tensor_copy(osum[mo][:, r * NT:(r + 1) * NT], ps[:])
                else:
                    ot = osb.tile([128, NT], F32, tag="osb", name="osb")
                    nc.vector.tensor_tensor(
                        ot[:], ps[:], osum[mo][:, r * NT:(r + 1) * NT],
                        mybir.AluOpType.add,
                    )
                    nc.sync.dma_start(out_d[mo, :, r * NT:(r + 1) * NT], ot[:])

            def att_unit(m, qi, grp2):
                """Both heads / 2 kj tiles: 4 score MMs (full 128-contraction
                against zero-padded K), 2 exps, 4 AV MMs."""
                qsl = slice(qi * NT, (qi + 1) * NT)
                sp = [spsum.tile([128, 2, 512], F32, tag="sps", name="sps")
                      for _ in range(2)]
                for j in range(2):
                    kjt = grp2 * 2 + j
                    ksl = slice(kjt * 128, (kjt + 1) * 128)
                    nc.tensor.matmul(sp[0][:, j, 0:NT], k0p[m][:, ksl],
                                     q_sb[m][:, qsl], start=True, stop=True)
                    nc.tensor.matmul(sp[1][:, j, 0:NT], k1p[m][:, ksl],
                                     q_sb[m][:, qsl], start=True, stop=True)
                ets = []
                for hh in range(2):
                    et = esb.tile([128, 2, NT], BF, tag="et", name="et")
                    nc.scalar.activation(et[:], sp[hh][:, :, 0:NT], EXP, scale=0.125)
                    ets.append(et)
                for hh in range(2):
                    h = 2 * m + hh
                    for j in range(2):
                        kjt = grp2 * 2 + j
                        nc.tensor.matmul(
                            av_cur[hh][0:65, :], vt_sb[h][:, kjt, 0:65],
                            ets[hh][:, j, :],
                            start=(kjt == 0), stop=(kjt == NKJ - 1),
                        )

            def normalize(m, qi, hh):
                avf = nsb.tile([128, NT], F32, tag="avf", name="avf")
                nc.vector.tensor_copy(avf[0:65, :], av_cur[hh][0:65, :])
                dn = nsb.tile([1, NT], F32, tag="dn", name="dn")
                nc.sync.dma_start(dn[:], avf[64:65, :])
                rc = nsb.tile([1, NT], F32, tag="rc", name="rc")
                nc.vector.reciprocal_approx_fast(rc[:], dn[:])
                rb = nsb.tile([64, NT], F32, tag="rb", name="rb")
                nc.gpsimd.partition_broadcast(rb[:], rc[:])
                dst = opad[m][64 * hh:64 * hh + 64,
                              qi * RT + 1: qi * RT + RT + 1, 1:49]
                nc.vector.tensor_tensor(
                    dst,
                    avf[0:64, :].rearrange("p (r c) -> p r c", c=W),
                    rb[:].rearrange("p (r c) -> p r c", c=W),
                    mybir.AluOpType.mult,
                )

            def run_attention(m, row_fillers, paced):
                """Emit all attention units for head-pair m.

                row_fillers: dict qi -> list of callables emitted at row start.
                paced: list of (mm_weight, callable) dripped across all units
                at a rate proportional to matmul count.
                """
                fi = 0
                n_units = NROW * 9
                total_w = sum(w for w, _ in paced) or 1
                done_w = 0
                ui = 0
                for qi in range(NROW):
                    av_cur[0] = apsum.tile([128, NT], F32, tag="avps", name="avps")
                    av_cur[1] = apsum.tile([128, NT], F32, tag="avps", name="avps")
                    for grp2 in range(9):
                        att_unit(m, qi, grp2)
                        # row fillers go two units in, so the previous row's
                        # normalize chain (which some fillers read) has landed
                        if grp2 == 1:
                            for f in row_fillers.get(qi, ()):
                                f()
                        ui += 1
                        while fi < len(paced) and done_w * n_units < ui * total_w:
                            w, f = paced[fi]
                            f()
                            done_w += w
                            fi += 1
                    for hh in range(2):
                        normalize(m, qi, hh)
                while fi < len(paced):
                    paced[fi][1]()
                    fi += 1

            av_cur = [None, None]

            # --- m0 attention.  Row fillers: next q-m0 rowtile.  Paced: the
            # full m1 conv pipeline (v, transposes, k, q-r0).
            rowf_b = {qi: [lambda r=qi + 1: conv_row(0, WQ, copy_plain(q_sb[0]),
                                                     r, fpsum, "fps")]
                      for qi in range(NROW - 1)}
            paced_b = (
                [(18, lambda r=r: conv_row(1, WV, copy_plain(v_sb[1]), r,
                                           fpsum, "fps"))
                 for r in range(NROW)]
                # transposes in contiguous batches of 9: the PE transpose mode
                # switch costs a drain, so don't drip them singly
                + [(9, lambda hh=hh, k0=k0: [
                        transpose_unit(1, hh, kt, fpsum, "fps")
                        for kt in range(k0, k0 + 9)])
                   for hh in range(2) for k0 in (0, 9)]
                + [(18, lambda r=r: conv_row(1, WK, copy_ksplit(1), r,
                                             fpsum, "fps"))
                   for r in range(NROW)]
                + [(18, lambda: conv_row(1, WQ, copy_plain(q_sb[1]), 0,
                                         fpsum, "fps"))]
            )
            run_attention(0, rowf_b, paced_b)

            # --- m1 attention.  Row fillers: next q-m1 rowtile, plus the
            # o-conv kg1 rows whose opad[1] inputs completed two rows ago.
            # Paced: the o-conv kg0 pass (opad[0] is fully available).
            rowf_c = {}
            for qi in range(NROW - 1):
                rowf_c[qi] = [lambda r=qi + 1: conv_row(1, WQ, copy_plain(q_sb[1]),
                                                        r, fpsum, "fps")]
            for qi in range(2, NROW):
                rowf_c.setdefault(qi, []).extend(
                    [lambda mo=mo, r=qi - 2: oconv_row_unit(mo, r, 1)
                     for mo in range(2)]
                )
            # r-outer so both kg0 partials for row r are emitted well before
            # the kg1 pass for row r (rowf_c at qi=r+2) reads osum.
            paced_c = [(9, lambda mo=mo, r=r: oconv_row_unit(mo, r, 0))
                       for r in range(NROW) for mo in range(2)]
            run_attention(1, rowf_c, paced_c)

            # tail: the last two kg1 o-conv rows (need the final normalizes)
            for r in (NROW - 2, NROW - 1):
                for mo in range(2):
                    oconv_row_unit(mo, r, 1)

    nc.compile()
    _cached = nc
    return nc


def make_in_maps(hidden_states, wq, wk, wv, wo):
    """Shard + pre-transform full inputs into 8 per-core input dicts.

    All device tensors are partition-major so each DMA is one descriptor
    per partition with a large contiguous line.
    """
    bf = ml_dtypes.bfloat16
    hidden_states = np.asarray(hidden_states, np.float32)
    in_maps = []
    for core in range(NCORES):
        b, g = core // 2, core % 2
        xp = np.zeros((C, 50, 50), np.float32)
        xp[:, 1:49, 1:49] = hidden_states[b]
        # [2, 128, 50, 50] -> [128, 2, 50, 50]
        xpad = np.ascontiguousarray(
            xp.reshape(2, 128, 50, 50).transpose(1, 0, 2, 3)
        ).astype(bf)
        # per matrix: [9, 2, 128p, 256o] -> [128p, 2m, 9, 2, 128o]; stack to
        # [128, 3, 2, 9, 2, 128]
        wstk = np.stack(
            [
                np.asarray(w, np.float32)[g * 256:(g + 1) * 256]
                .transpose(2, 3, 1, 0)
                .reshape(9, 2, 128, 2, 128)
                .transpose(2, 3, 0, 1, 4)
                for w in (wq, wk, wv)
            ],
            axis=1,
        )
        wstk = np.ascontiguousarray(wstk).astype(bf)
        # [9, 2, 128, 256] -> [128, 9, 2, 256]
        wog = (
            np.asarray(wo, np.float32)[:, g * 256:(g + 1) * 256]
            .transpose(2, 3, 1, 0)
            .reshape(9, 2, 128, 256)
        )
        wog = np.ascontiguousarray(wog.transpose(2, 0, 1, 3)).astype(bf)
        in_maps.append({"xpad": xpad, "wqkv": wstk, "wo": wog})
    return in_maps


def combine_outputs(per_core_outs):
    """Sum the two head-group partials per batch sample."""
    out = np.empty((B, C, H, W), np.float32)
    for b in range(B):
        acc = per_core_outs[2 * b].reshape(C, HW).astype(np.float32) + \
              per_core_outs[2 * b + 1].reshape(C, HW).astype(np.float32)
        out[b] = acc.reshape(C, H, W)
    return out


def kernel(hidden_states, wq, wk, wv, wo):
    from concourse.bass_utils import run_bass_kernel_spmd

    nc = _build()
    in_maps = make_in_maps(hidden_states, wq, wk, wv, wo)
    res = run_bass_kernel_spmd(nc, in_maps, core_ids=list(range(NCORES)))
    return combine_outputs([r["out"] for r in res.results])
